# revision 1
# baseline (speedup 1.0000x reference)
"""AConnect (nn_AConnect_82368882803074) Trainium2 kernel, v2.

Reference computation:
    memW[b]    = W * Werr_bank[idx[b]]             [B, D_in, D_out]
    membias[b] = bias * Berr_bank[idx[b]]          [B, 1, D_out]
    Z[b]       = X[b] @ memW[b] + membias[b]       [B, D_out]

Strategy: data-parallel over the batch across 8 NeuronCores with global
bank dedup. The host groups samples by bank index and packs one bank per
"slot" (up to M=4 samples ride along as extra matmul columns); slots are
spread over the 8 cores. The host only moves/casts data (gather,
transpose, bf16 cast, zero-padding, output permutation); all arithmetic
(W ⊙ E, X @ (W ⊙ E), bias ⊙ Berr and the final add) runs on device.

Changes vs v1 (v1: 94.9 us harness / 111.8 us traced; now ~75-82 us traced,
run-to-run HAM-phase variance is ~+/-4 us):
- Banks are cast to bf16 on the host, halving HBM read traffic (the
  dominant cost) from ~29 MB to ~15 MB per core; W/X upload bf16 too, so
  no device-side casts remain and SWDGE cast-DMA (1.8 us/transfer gpsimd
  ucode, ~5 us engine-library warmup) is not needed.
- Bank loads are paired (1 MB per DMA) and alternate between the two
  HWDGE rings (sync + scalar engines), each ring moving ~360 GB/s; the
  gpsimd SWDGE ring measured ~6 us between issues and is not used.
- Per-slot PSUM drains (ScalarE, [4, 1024] per slot pair) write one
  staging tile; a single rearranging store at the end replaces v1's
  per-pair output DMAs.
- VectorE runs exactly one [128, 2048] bf16 multiply per slot (2x mode,
  ~1.22 us) and nothing else — membias moved to gpsimd, W-doubling and
  all casts moved to the host.
- The per-slot k=1 bias matmul is kept even for zero bias: removing it
  measured ~5 us SLOWER — the extra 512-col streams hold the PE's HAM
  activity monitor at 2.4 GHz through VectorE supply gaps (default PE
  state is K=4/8 half-clock; >~5 us idle re-throttles).
"""

import numpy as np

B, D_IN, D_OUT, N_BANK, N_CORES = 256, 512, 512, 1000, 8
P = 128  # partitions
C = D_IN // P  # 4 k-chunks
M = 4  # samples per bank slot (max observed bank multiplicity is 3)
NWARM = 10  # PE warm-up matmuls (HAM throttle release)

_CACHE = {}
last_exec_time_ns = None


def _build_nc(K, zero_bias):
    """Device graph for K bank-slots per core (K even; padded on host).

    zero_bias=True omits the membias path entirely (bias input is all
    zeros, as produced by the reference setup); the general graph keeps
    it via the per-slot k=1 bias matmul."""
    import concourse.mybir as mybir
    import concourse.tile as tile
    from concourse import bacc

    f32 = mybir.dt.float32
    bf16 = mybir.dt.bfloat16
    nc = bacc.Bacc()

    R = K * M  # output rows, slot-major: row t*M + j = slot t, column j
    K2 = K // 2  # load pairs
    W2 = 2 * C * D_OUT  # free elems per pair tile (2 slots)
    # eg2[q, p, u*2048 + c*512 + o] = E[2q+u, c*128+p, o] (bf16, host layout)
    eg2 = nc.dram_tensor("eg2", [K2, P, W2], bf16, kind="ExternalInput")
    wt = nc.dram_tensor("wt", [P, C * D_OUT], bf16, kind="ExternalInput")
    xtt = nc.dram_tensor("xtt", [P, C * R], bf16, kind="ExternalInput")
    if not zero_bias:
        bb = nc.dram_tensor("bb", [K, D_OUT], f32, kind="ExternalInput")
        beg = nc.dram_tensor("beg", [K, D_OUT], f32, kind="ExternalInput")
    out = nc.dram_tensor("out", [R, D_OUT], f32, kind="ExternalOutput")

    with tile.TileContext(nc) as tc:
        with (
            tc.tile_pool(name="const", bufs=1) as constp,
            tc.tile_pool(name="ep", bufs=6) as ep,
            tc.tile_pool(name="wep", bufs=4) as wep,
            tc.tile_pool(name="ps", bufs=3, space="PSUM") as psp,
            tc.tile_pool(name="scr", bufs=2) as scr,
        ):
            # Resident operands (already bf16 from host; scalar=Activation
            # HWDGE ring carries the small loads, W first so the multiply
            # pipeline can start as soon as the first bank pair lands).
            w_b = constp.tile([P, C * D_OUT], bf16, name="wb")
            nc.scalar.dma_start(w_b[:], wt[:])
            x_b = constp.tile([P, C * R], bf16)
            nc.scalar.dma_start(x_b[:], xtt[:])

            if not zero_bias:
                # membias = bias * Berr[bank], one row per bank slot, in
                # bf16: it joins the PE accumulation via a k=1 matmul below,
                # which needs the rhs at partition 0 (single reshape DMA).
                bias_k = scr.tile([K, D_OUT], f32, name="bias_k", tag="bq")
                nc.scalar.dma_start(bias_k[:], bb[:])
                berr_k = scr.tile([K, D_OUT], f32, name="berr_k", tag="eq")
                nc.scalar.dma_start(berr_k[:], beg[:])
                # mbk runs on VectorE (first in its queue, ~0.9 us): on
                # gpsimd it starts ~18 us in, and the mbrow DMA below blocks
                # the scalar engine queue — and with it every scalar-ring
                # pair load — until mbk completes.
                mbk = constp.tile([K, D_OUT], bf16, name="mbk")
                nc.vector.tensor_mul(mbk[:], bias_k[:], berr_k[:])
                mbrow = constp.tile([1, K * D_OUT], bf16)
                nc.scalar.dma_start(mbrow[:], mbk[:])
                ones_b = constp.tile([1, M], bf16)
                nc.any.memset(ones_b[:], 1.0)

            warm = psp.tile([M, D_OUT], f32, name="warm", bufs=1)

            # Dummy matmuls on resident tiles release the PE's HAM throttle
            # (default state is K=4/8 half-clock; ~3.4us of activity frees it)
            for _ in range(NWARM):
                nc.tensor.matmul(
                    warm[:], x_b[:, 0:M], w_b[:, 0:D_OUT], start=True, stop=True
                )

            # output staging at partition base 0: osb[j, t*512+n] = Z[t*M+j, n]
            osb = constp.tile([M, K * D_OUT], f32, name="osb")
            # alternate the 1 MB pair loads over the two HWDGE rings (the
            # gpsimd SWDGE ring issues loads ~6us apart — too slow to help)
            def load_ring(q):
                return (nc.sync, nc.scalar)[q % 2]

            H = C * D_OUT
            for q in range(K2):
                ebp = ep.tile([P, W2], bf16)
                if q == 0:
                    # first pair split in two halves (both on the sync ring:
                    # a half on the scalar ring delays that whole queue) so
                    # the first multiply's region dep fires ~1.4us after the
                    # first half lands instead of after the full 1 MB
                    nc.sync.dma_start(ebp[:, 0:H], eg2[0, :, 0:H])
                    nc.sync.dma_start(ebp[:, H:W2], eg2[0, :, H:W2])
                else:
                    load_ring(q).dma_start(ebp[:], eg2[q])
                wep_t = wep.tile([P, W2], bf16)
                ps = psp.tile([M, 2 * D_OUT], f32)  # slot pair: 2 PSUM banks
                for u in range(2):
                    t = 2 * q + u
                    nc.vector.tensor_mul(
                        wep_t[:, u * C * D_OUT : (u + 1) * C * D_OUT],
                        ebp[:, u * C * D_OUT : (u + 1) * C * D_OUT],
                        w_b[:],
                    )
                    for c in range(C):
                        nc.tensor.matmul(
                            ps[:, u * D_OUT : (u + 1) * D_OUT],
                            x_b[:, (c * K + t) * M : (c * K + t) * M + M],
                            wep_t[
                                :,
                                u * C * D_OUT + c * D_OUT : u * C * D_OUT
                                + (c + 1) * D_OUT,
                            ],
                            start=(c == 0),
                            stop=(zero_bias and c == C - 1),
                        )
                    if not zero_bias:
                        # bias joins the PSUM accumulation: the k=1 matmul
                        # ones[1,M]^T @ mb[1,N] broadcasts the bank's membias
                        # row onto all M output rows
                        nc.tensor.matmul(
                            ps[:, u * D_OUT : (u + 1) * D_OUT],
                            ones_b[:],
                            mbrow[0:1, t * D_OUT : (t + 1) * D_OUT],
                            start=False,
                            stop=True,
                        )
                # drain the pair on the otherwise idle ScalarE into the
                # staging tile; one store at the end
                nc.scalar.copy(
                    osb[0:M, 2 * q * D_OUT : 2 * (q + 1) * D_OUT], ps[:]
                )

            nc.sync.dma_start(
                out[:].rearrange("(t j) n -> j t n", j=M),
                osb[:].rearrange("j (t n) -> j t n", n=D_OUT),
            )

    nc.compile()
    return nc


def _pack(idx):
    """Group samples by bank, pack banks onto cores.

    Returns (K, plan) where plan[c] is a list of (bank, [samples]) slots,
    each slot carrying at most M samples of one bank. K is even.
    """
    from collections import defaultdict

    groups = defaultdict(list)
    for s, b in enumerate(idx):
        groups[int(b)].append(s)
    # one slot per <=M samples of a bank
    slots = []
    for b, ss in groups.items():
        for i in range(0, len(ss), M):
            slots.append((b, ss[i : i + M]))
    slots.sort(key=lambda x: -len(x[1]))
    plan = [[] for _ in range(N_CORES)]
    for b, ss in slots:
        c = min(range(N_CORES), key=lambda c: len(plan[c]))
        plan[c].append((b, ss))
    K = max(len(p) for p in plan)
    K += K % 2  # pair loads need an even slot count
    return K, plan


def _install_trace_shim():
    """Register the axon NTFF profile hook bass_utils expects (the agent
    image lacks antenv.axon_hooks; the C ABI is in libaxon_pjrt.so)."""
    import contextlib
    import ctypes
    import sys
    import types

    if "antenv.axon_hooks" in sys.modules:
        return
    lib = ctypes.CDLL("/opt/axon/libaxon_pjrt.so")
    if not hasattr(lib, "axon_start_nrt_profile"):
        hook = None
    else:
        lib.axon_start_nrt_profile.argtypes = [
            ctypes.POINTER(ctypes.c_int64),
            ctypes.c_size_t,
        ]
        lib.axon_start_nrt_profile.restype = ctypes.c_int64
        lib.axon_stop_nrt_profile.argtypes = [ctypes.c_char_p]
        lib.axon_stop_nrt_profile.restype = ctypes.c_int64

        @contextlib.contextmanager
        def hook(output_dir, device_ids):
            import jax

            jax.devices()
            if device_ids:
                ids = (ctypes.c_int64 * len(device_ids))(*device_ids)
                rc = lib.axon_start_nrt_profile(ids, len(device_ids))
            else:
                rc = lib.axon_start_nrt_profile(None, 0)
            if rc != 0:
                raise RuntimeError(f"axon_start_nrt_profile rc={rc}")
            try:
                yield
            finally:
                n = lib.axon_stop_nrt_profile(str(output_dir).encode())
                print(f"ntff profile: {n} file(s) -> {output_dir}", file=sys.stderr)

    mod = types.ModuleType("antenv.axon_hooks")
    mod.get_axon_ntff_profile_hook = lambda: hook
    mod.set_axon_ntff_profile_hook = lambda h: None
    sys.modules["antenv.axon_hooks"] = mod


def kernel(X, W, bias, Werr_bank, Berr_bank, idx):
    global last_exec_time_ns
    import os

    import ml_dtypes

    from concourse.bass_utils import run_bass_kernel_spmd

    bf16 = ml_dtypes.bfloat16
    X = np.asarray(X, dtype=np.float32)
    W = np.asarray(W, dtype=np.float32)
    bias = np.asarray(bias, dtype=np.float32)
    Werr_bank = np.asarray(Werr_bank, dtype=np.float32)
    Berr_bank = np.asarray(Berr_bank, dtype=np.float32)
    idx = np.asarray(idx, dtype=np.int32)

    K, plan = _pack(idx)
    # NB: a zero-bias specialization (dropping the per-slot bias matmul)
    # measures ~5us SLOWER despite less PE work — the 512-col bias matmuls
    # double as HAM activity keepers that hold the PE at 2.4 GHz through
    # VectorE supply gaps. Keep the bias path unconditionally.
    zero_bias = False
    if ("nc", K, zero_bias) not in _CACHE:
        _CACHE[("nc", K, zero_bias)] = _build_nc(K, zero_bias)
    nc = _CACHE[("nc", K, zero_bias)]
    R = K * M
    K2 = K // 2

    # Host-side sharding / layout (pure data movement + dtype cast).
    wt = np.ascontiguousarray(
        W.astype(bf16).reshape(C, P, D_OUT).transpose(1, 0, 2).reshape(P, C * D_OUT)
    )
    bb = np.ascontiguousarray(np.broadcast_to(bias.reshape(1, D_OUT), (K, D_OUT)))

    in_maps = []
    row_of_sample = np.full(B, -1, dtype=np.int64)  # (core, row) flattened
    for c_id in range(N_CORES):
        slots = plan[c_id]
        banks = [b for b, _ in slots] + [0] * (K - len(slots))
        eg = Werr_bank[banks].astype(bf16)  # [K, D_in, D_out] bf16
        # pair-interleaved layout: eg2[q, p, u*2048 + c*512 + o]
        eg2 = np.ascontiguousarray(
            eg.reshape(K2, 2, C, P, D_OUT)
            .transpose(0, 3, 1, 2, 4)
            .reshape(K2, P, 2 * C * D_OUT)
        )
        # X columns and output rows in slot-major order: row t*M + j
        xs = np.zeros((R, D_IN), dtype=np.float32)
        beg = np.ascontiguousarray(Berr_bank[banks, 0, :])  # [K, D_out]
        for t, (b, ss) in enumerate(slots):
            for j, s in enumerate(ss):
                xs[t * M + j] = X[s]
                row_of_sample[s] = c_id * R + t * M + j
        xtt = np.ascontiguousarray(
            xs.T.astype(bf16).reshape(C, P, R).transpose(1, 0, 2).reshape(P, C * R)
        )
        m = {"eg2": eg2, "wt": wt, "xtt": xtt}
        if not zero_bias:
            m["bb"] = bb
            m["beg"] = beg
        in_maps.append(m)
    assert (row_of_sample >= 0).all()

    trace = os.environ.get("BASS_KERNEL_TRACE") == "1"
    if trace:
        _install_trace_shim()
    res = run_bass_kernel_spmd(
        nc,
        in_maps,
        core_ids=list(range(N_CORES)),
        trace=trace,
        trace_cores=(
            list(range(N_CORES))
            if os.environ.get("BASS_KERNEL_TRACE_ALL") == "1"
            else [0]
        )
        if trace
        else None,
    )
    last_exec_time_ns = res.exec_time_ns
    allrows = np.concatenate([r["out"] for r in res.results], axis=0)  # [8*R, 512]
    return np.ascontiguousarray(allrows[row_of_sample])



# revision 6
# speedup vs baseline: 1.0157x; 1.0157x over previous
"""AConnect (nn_AConnect_82368882803074) Trainium2 kernel, v3.

Reference computation:
    memW[b]    = W * Werr_bank[idx[b]]             [B, D_in, D_out]
    membias[b] = bias * Berr_bank[idx[b]]          [B, 1, D_out]
    Z[b]       = X[b] @ memW[b] + membias[b]       [B, D_out]

Strategy: data-parallel over the batch across 8 NeuronCores with global
bank dedup. The host groups samples by bank index into one "slot" per
bank (up to M=4 samples ride along as extra matmul columns) and spreads
slots over the 8 cores (K=28 slots/core for the reference idx). The
host only moves/casts data (gather, transpose, bf16 cast, padding,
output permutation); all arithmetic (W ⊙ E, X @ (W ⊙ E), and the bias
path when nonzero) runs on device.

v3 changes vs v2 (80.3 us traced):
- Slot-granular 0.5 MiB bank loads alternating between the two HWDGE
  rings (sync + scalar), 4 KB packets either way; deep ep pool so a
  drain or issue stall never starves the rings.
- PE col-tiling: slot t -> column group j=t%3 (tile_position (0, 32j)),
  so the 3 slots of a group run CONCURRENTLY in the 128x128 array
  (~2.4-3x PE throughput; quadrant 3 is unusable per HW bug). Matmuls
  are interleaved c-major inside each group to keep 3 streams alive.
- Outputs accumulate at PSUM partitions 32j..32j+4 of a per-group bank;
  ScalarE drains each slot [4, 512] into a packed SBUF staging tile;
  six plain contiguous DMAs (2 halves x 3 j-strips) replace v2's single
  rearranging store whose strided 4-partition descriptors ran at
  ~20 GB/s and made an ~11 us tail.
- W ⊙ E multiplies split across DVE (2x_1p, ~1.19 us/slot) and the
  otherwise-idle GpSimd for a few slots to pull DVE off the critical
  path.
- bias == 0 for this model (reference setup): the membias path is
  compiled only when the host sees a nonzero bias * Berr product.
- PE HAM warm-up runs on memset dummies instead of waiting for the
  W/X loads, releasing the half-clock throttle before real work.
"""

import numpy as np

B, D_IN, D_OUT, N_BANK, N_CORES = 256, 512, 512, 1000, 8
P = 128  # partitions
C = D_IN // P  # 4 k-chunks
M = 4  # samples per bank slot (max observed bank multiplicity is 3)
NWARM = 8  # PE warm-up matmuls (HAM throttle release)
NGP = 4  # slots whose W*E multiply runs on GpSimd instead of DVE

_CACHE = {}
last_exec_time_ns = None


def _gp_slots(K):
    """Slots whose multiply runs on gpsimd: spread mid-run, never the
    first two (latency-critical ramp) nor the last (tail latency)."""
    if NGP <= 0 or K <= 6:
        return set()
    step = max(1, (K - 3) // NGP)
    return set(list(range(2, K - 1, step))[:NGP])


def _build_nc(K, with_bias):
    """Device graph for K bank-slots per core."""
    import concourse.mybir as mybir
    import concourse.tile as tile
    from concourse import bacc

    f32 = mybir.dt.float32
    bf16 = mybir.dt.bfloat16
    nc = bacc.Bacc()

    G = -(-K // 3)  # PE column groups of up to 3 slots
    R = K * M
    H = C * D_OUT  # 2048 free elems per slot tile
    # eg[t, p, c*512 + o] = E[banks[t], c*128+p, o]  (bf16, host layout)
    eg = nc.dram_tensor("eg", [K, P, H], bf16, kind="ExternalInput")
    wt = nc.dram_tensor("wt", [P, H], bf16, kind="ExternalInput")
    xtt = nc.dram_tensor("xtt", [P, C * R], bf16, kind="ExternalInput")
    if with_bias:
        bb = nc.dram_tensor("bb", [K, D_OUT], f32, kind="ExternalInput")
        beg = nc.dram_tensor("beg", [K, D_OUT], f32, kind="ExternalInput")
    # out[j, m, g, o] = Z[slot 3g+j, sample m][o]
    out = nc.dram_tensor("out", [3, M, G, D_OUT], f32, kind="ExternalOutput")

    gp = _gp_slots(K)

    with tile.TileContext(nc) as tc:
        with (
            tc.tile_pool(name="const", bufs=1) as constp,
            tc.tile_pool(name="ep", bufs=14) as ep,
            tc.tile_pool(name="wep", bufs=6) as wep,
            tc.tile_pool(name="ps", bufs=4, space="PSUM") as psp,
        ):
            # HAM warm-up on memset dummies (no DMA dependency): ~3.4us of
            # PE activity releases the default K=4/8 half-clock state.
            dum = constp.tile([P, M + D_OUT], bf16, name="dum")
            nc.gpsimd.memset(dum[:], 1.0)
            warmps = psp.tile([M, D_OUT], f32, name="warm", bufs=1)
            for _ in range(NWARM):
                nc.tensor.matmul(
                    warmps[:], dum[:, 0:M], dum[:, M:], start=True, stop=True
                )

            # Resident operands (bf16 from host). W first on the scalar ring
            # so the first multiply can start as soon as slot 0 lands.
            w_b = constp.tile([P, H], bf16, name="wb")
            nc.scalar.dma_start(w_b[:], wt[:])
            x_b = constp.tile([P, C * R], bf16, name="xb")
            nc.scalar.dma_start(x_b[:], xtt[:])
            if with_bias:
                bias_k = constp.tile([K, D_OUT], f32, name="bias_k")
                nc.scalar.dma_start(bias_k[:], bb[:])
                berr_k = constp.tile([K, D_OUT], f32, name="berr_k")
                nc.scalar.dma_start(berr_k[:], beg[:])
                mbk = constp.tile([K, D_OUT], bf16, name="mbk")
                nc.vector.tensor_mul(mbk[:], bias_k[:], berr_k[:])
                mbrow = constp.tile([1, K * D_OUT], bf16, name="mbrow")
                nc.scalar.dma_start(mbrow[:], mbk[:])
                ones_b = constp.tile([1, M], bf16, name="ones")
                nc.gpsimd.memset(ones_b[:], 1.0)

            # Output staging: osb[32j+m, g*512+o] = Z[slot 3g+j, m][o]
            osb = constp.tile([96, G * D_OUT], f32, name="osb")

            def load_ring(t):
                return (nc.sync, nc.scalar)[t % 2]

            ghalf = G // 2
            wep_tiles = []
            for t in range(K):
                g, j = t // 3, t % 3
                nslot = min(3, K - g * 3)
                ebt = ep.tile([P, H], bf16)
                if t == 0:
                    # split the first load+multiply so the first matmuls'
                    # region deps fire after 0.25 MiB instead of 0.5 MiB
                    nc.sync.dma_start(ebt[:, 0 : H // 2], eg[0, :, 0 : H // 2])
                    nc.sync.dma_start(ebt[:, H // 2 : H], eg[0, :, H // 2 : H])
                else:
                    load_ring(t).dma_start(ebt[:], eg[t])

                wep_t = wep.tile([P, H], bf16)
                mul_eng = nc.gpsimd if t in gp else nc.vector
                if t == 0:
                    mul_eng.tensor_mul(
                        wep_t[:, 0 : H // 2], ebt[:, 0 : H // 2], w_b[:, 0 : H // 2]
                    )
                    mul_eng.tensor_mul(
                        wep_t[:, H // 2 : H], ebt[:, H // 2 : H], w_b[:, H // 2 : H]
                    )
                else:
                    mul_eng.tensor_mul(wep_t[:], ebt[:], w_b[:])
                wep_tiles.append(wep_t)

                if j != nslot - 1:
                    continue

                # interleave the group's matmuls c-major so its (up to) 3
                # column-group streams run concurrently in the PE array
                ps = psp.tile([P, D_OUT], f32)
                for c in range(C):
                    for jj in range(nslot):
                        tt = g * 3 + jj
                        nc.tensor.matmul(
                            ps[32 * jj : 32 * jj + M, :],
                            x_b[:, (c * K + tt) * M : (c * K + tt) * M + M],
                            wep_tiles[jj][:, c * D_OUT : (c + 1) * D_OUT],
                            start=(c == 0),
                            stop=(not with_bias and c == C - 1),
                        )
                if with_bias:
                    for jj in range(nslot):
                        tt = g * 3 + jj
                        nc.tensor.matmul(
                            ps[32 * jj : 32 * jj + M, :],
                            ones_b[:],
                            mbrow[0:1, tt * D_OUT : (tt + 1) * D_OUT],
                            start=False,
                            stop=True,
                        )
                # drain the group on the otherwise idle ScalarE
                for jj in range(nslot):
                    nc.scalar.copy(
                        osb[32 * jj : 32 * jj + M, g * D_OUT : (g + 1) * D_OUT],
                        ps[32 * jj : 32 * jj + M, :],
                    )
                wep_tiles = []

                # store the first half of the staging tile mid-run
                if g == ghalf - 1:
                    for jj in range(3):
                        nc.sync.dma_start(
                            out[jj, :, 0:ghalf, :],
                            osb[32 * jj : 32 * jj + M, 0 : ghalf * D_OUT],
                        )
            for jj in range(3):
                load_ring(jj).dma_start(
                    out[jj, :, ghalf:G, :],
                    osb[32 * jj : 32 * jj + M, ghalf * D_OUT : G * D_OUT],
                )

    nc.compile()
    return nc


def _pack(idx):
    """Group samples by bank, pack bank-slots onto cores.

    Returns (K, plan): plan[c] is a list of (bank, [samples]) slots, each
    carrying at most M samples of one bank; K = max slots per core.
    """
    from collections import defaultdict

    groups = defaultdict(list)
    for s, b in enumerate(idx):
        groups[int(b)].append(s)
    slots = []
    for b, ss in groups.items():
        for i in range(0, len(ss), M):
            slots.append((b, ss[i : i + M]))
    slots.sort(key=lambda x: -len(x[1]))
    plan = [[] for _ in range(N_CORES)]
    for b, ss in slots:
        c = min(range(N_CORES), key=lambda c: len(plan[c]))
        plan[c].append((b, ss))
    K = max(len(p) for p in plan)
    return K, plan


def _install_trace_shim():
    """Register the axon NTFF profile hook bass_utils expects (the agent
    image lacks antenv.axon_hooks; the C ABI is in libaxon_pjrt.so)."""
    import contextlib
    import ctypes
    import sys
    import types

    if "antenv.axon_hooks" in sys.modules:
        return
    lib = ctypes.CDLL("/opt/axon/libaxon_pjrt.so")
    if not hasattr(lib, "axon_start_nrt_profile"):
        hook = None
    else:
        lib.axon_start_nrt_profile.argtypes = [
            ctypes.POINTER(ctypes.c_int64),
            ctypes.c_size_t,
        ]
        lib.axon_start_nrt_profile.restype = ctypes.c_int64
        lib.axon_stop_nrt_profile.argtypes = [ctypes.c_char_p]
        lib.axon_stop_nrt_profile.restype = ctypes.c_int64

        @contextlib.contextmanager
        def hook(output_dir, device_ids):
            import jax

            jax.devices()
            if device_ids:
                ids = (ctypes.c_int64 * len(device_ids))(*device_ids)
                rc = lib.axon_start_nrt_profile(ids, len(device_ids))
            else:
                rc = lib.axon_start_nrt_profile(None, 0)
            if rc != 0:
                raise RuntimeError(f"axon_start_nrt_profile rc={rc}")
            try:
                yield
            finally:
                n = lib.axon_stop_nrt_profile(str(output_dir).encode())
                print(f"ntff profile: {n} file(s) -> {output_dir}", file=sys.stderr)

    mod = types.ModuleType("antenv.axon_hooks")
    mod.get_axon_ntff_profile_hook = lambda: hook
    mod.set_axon_ntff_profile_hook = lambda h: None
    sys.modules["antenv.axon_hooks"] = mod


def kernel(X, W, bias, Werr_bank, Berr_bank, idx):
    global last_exec_time_ns
    import os

    import ml_dtypes

    from concourse.bass_utils import run_bass_kernel_spmd

    bf16 = ml_dtypes.bfloat16
    X = np.asarray(X, dtype=np.float32)
    W = np.asarray(W, dtype=np.float32)
    bias = np.asarray(bias, dtype=np.float32)
    Werr_bank = np.asarray(Werr_bank, dtype=np.float32)
    Berr_bank = np.asarray(Berr_bank, dtype=np.float32)
    idx = np.asarray(idx, dtype=np.int32)

    K, plan = _pack(idx)
    G = -(-K // 3)
    R = K * M
    with_bias = bool(np.any(bias))
    if ("nc", K, with_bias) not in _CACHE:
        _CACHE[("nc", K, with_bias)] = _build_nc(K, with_bias)
    nc = _CACHE[("nc", K, with_bias)]

    wt = np.ascontiguousarray(
        W.astype(bf16).reshape(C, P, D_OUT).transpose(1, 0, 2).reshape(P, C * D_OUT)
    )

    in_maps = []
    row_of_sample = np.full(B, -1, dtype=np.int64)
    for c_id in range(N_CORES):
        slots = plan[c_id]
        banks = [b for b, _ in slots] + [0] * (K - len(slots))
        eg = np.ascontiguousarray(
            Werr_bank[banks]
            .astype(bf16)
            .reshape(K, C, P, D_OUT)
            .transpose(0, 2, 1, 3)
            .reshape(K, P, C * D_OUT)
        )
        xs = np.zeros((R, D_IN), dtype=np.float32)
        for t, (b, ss) in enumerate(slots):
            g, j = t // 3, t % 3
            for m, s in enumerate(ss):
                xs[t * M + m] = X[s]
                # device row (j, m, g) of out [3, M, G, 512]
                row_of_sample[s] = c_id * (3 * M * G) + (j * M + m) * G + g
        xtt = np.ascontiguousarray(
            xs.T.astype(bf16).reshape(C, P, R).transpose(1, 0, 2).reshape(P, C * R)
        )
        im = {"eg": eg, "wt": wt, "xtt": xtt}
        if with_bias:
            im["bb"] = np.ascontiguousarray(
                np.broadcast_to(bias.reshape(1, D_OUT), (K, D_OUT))
            )
            im["beg"] = np.ascontiguousarray(Berr_bank[banks, 0, :])
        in_maps.append(im)
    assert (row_of_sample >= 0).all()

    trace = os.environ.get("BASS_KERNEL_TRACE") == "1"
    if trace:
        _install_trace_shim()
    res = run_bass_kernel_spmd(
        nc,
        in_maps,
        core_ids=list(range(N_CORES)),
        trace=trace,
        trace_cores=(
            list(range(N_CORES))
            if os.environ.get("BASS_KERNEL_TRACE_ALL") == "1"
            else [0]
        )
        if trace
        else None,
    )
    last_exec_time_ns = res.exec_time_ns
    allrows = np.concatenate(
        [r["out"].reshape(3 * M * G, D_OUT) for r in res.results], axis=0
    )
    return np.ascontiguousarray(allrows[row_of_sample])


# revision 13
# speedup vs baseline: 1.2019x; 1.1833x over previous
"""AConnect (nn_AConnect_82368882803074) Trainium2 kernel, v3.

Reference computation:
    memW[b]    = W * Werr_bank[idx[b]]             [B, D_in, D_out]
    membias[b] = bias * Berr_bank[idx[b]]          [B, 1, D_out]
    Z[b]       = X[b] @ memW[b] + membias[b]       [B, D_out]

Strategy: data-parallel over the batch across 8 NeuronCores with global
bank dedup. The host groups samples by bank index into one "slot" per
bank (up to M=4 samples ride along as extra matmul columns) and spreads
slots over the 8 cores (K=28 slots/core for the reference idx). The
host only moves/casts data (gather, transpose, bf16 cast, padding,
output permutation); all arithmetic (W ⊙ E, X @ (W ⊙ E), and the bias
path when nonzero) runs on device.

v4 changes (v2: 80.3 us; v3: 79.1 us traced):
- Slot-granular 0.5 MiB bank loads alternating between the two HWDGE
  rings; the first PREFETCH load issues are emitted BEFORE any drain in
  program order, so an in-order engine queue waiting on a PSUM drain
  can never starve its DMA ring (v3's scalar ring sat blocked behind
  drain semaphores and spread 7.6 MiB over 65 us).
- PE col-tiling by slot pairs: slot t -> column group j=t%2 (PSUM
  partition base 32j), matmuls c-interleaved across the pair so two
  streams run concurrently in the array. With 2 concurrent streams the
  PE meets the DMA cadence even in the HAM half-clock state (v3's
  3-group bursts were too short to ever re-earn full clock, and its
  3-slot wep barrier put slow gpsimd multiplies on the critical path).
- All W ⊙ E multiplies on DVE (2x_1p): 28 x ~1.2 us < DMA 39 us. v3's
  gpsimd multiplies contended with DVE on the shared w_b tile reads and
  slowed every DVE op to 1.73 us.
- Outputs accumulate in PSUM [64, 512] per pair; ScalarE drains each
  slot [4, 512] into a packed staging tile; four plain contiguous DMAs
  (2 halves x 2 j-strips) replace v2's single rearranging store whose
  strided 4-partition descriptors ran at ~20 GB/s (an ~11 us tail).
- bias == 0 for this model (reference setup): the membias path is
  compiled only when the host sees a nonzero bias.
- PE HAM warm-up runs on memset dummies instead of waiting for the
  W/X loads, releasing the half-clock throttle before real work.
"""

import numpy as np

B, D_IN, D_OUT, N_BANK, N_CORES = 256, 512, 512, 1000, 8
P = 128  # partitions
C = D_IN // P  # 4 k-chunks
M = 4  # samples per bank slot (max observed bank multiplicity is 3)
NWARM = 8  # PE warm-up matmuls (HAM throttle release)
PREFETCH = 12  # load issues emitted ahead of the compute loop

_CACHE = {}
last_exec_time_ns = None


def _build_nc(K, with_bias):
    """Device graph for K bank-slots per core."""
    import concourse.mybir as mybir
    import concourse.tile as tile
    from concourse import bacc

    f32 = mybir.dt.float32
    bf16 = mybir.dt.bfloat16
    nc = bacc.Bacc()

    Q = -(-K // 2)  # PE pair groups (col-tiling over 2 column groups)
    R = K * M
    H = C * D_OUT  # 2048 free elems per slot tile
    # eg[t, p, c*512 + o] = E[banks[t], c*128+p, o]  (bf16, host layout)
    eg = nc.dram_tensor("eg", [K, P, H], bf16, kind="ExternalInput")
    wt = nc.dram_tensor("wt", [P, H], bf16, kind="ExternalInput")
    xtt = nc.dram_tensor("xtt", [P, C * R], bf16, kind="ExternalInput")
    if with_bias:
        bb = nc.dram_tensor("bb", [K, D_OUT], f32, kind="ExternalInput")
        beg = nc.dram_tensor("beg", [K, D_OUT], f32, kind="ExternalInput")
    # out[j, m, q, o] = Z[slot 2q+j, sample m][o]
    out = nc.dram_tensor("out", [2, M, Q, D_OUT], f32, kind="ExternalOutput")

    with tile.TileContext(nc) as tc:
        with (
            tc.tile_pool(name="const", bufs=1) as constp,
            tc.tile_pool(name="ep", bufs=14) as ep,
            tc.tile_pool(name="wep", bufs=6) as wep,
            tc.tile_pool(name="ps", bufs=4, space="PSUM") as psp,
        ):
            # HAM warm-up on memset dummies (no DMA dependency): ~3.4us of
            # PE activity releases the default K=4/8 half-clock state.
            dum = constp.tile([P, M + D_OUT], bf16, name="dum")
            nc.gpsimd.memset(dum[:], 1.0)
            warmps = psp.tile([M, D_OUT], f32, name="warm", bufs=1)
            for _ in range(NWARM):
                nc.tensor.matmul(
                    warmps[:], dum[:, 0:M], dum[:, M:], start=True, stop=True
                )

            # Resident operands (bf16 from host). W first on the scalar ring
            # so the first multiply can start as soon as slot 0 lands.
            w_b = constp.tile([P, H], bf16, name="wb")
            nc.scalar.dma_start(w_b[:], wt[:])
            x_b = constp.tile([P, C * R], bf16, name="xb")
            nc.scalar.dma_start(x_b[:], xtt[:])
            if with_bias:
                bias_k = constp.tile([K, D_OUT], f32, name="bias_k")
                nc.scalar.dma_start(bias_k[:], bb[:])
                berr_k = constp.tile([K, D_OUT], f32, name="berr_k")
                nc.scalar.dma_start(berr_k[:], beg[:])
                mbk = constp.tile([K, D_OUT], bf16, name="mbk")
                nc.vector.tensor_mul(mbk[:], bias_k[:], berr_k[:])
                mbrow = constp.tile([1, K * D_OUT], bf16, name="mbrow")
                nc.scalar.dma_start(mbrow[:], mbk[:])
                ones_b = constp.tile([1, M], bf16, name="ones")
                nc.gpsimd.memset(ones_b[:], 1.0)

            # Output staging: osb[32j+m, q*512+o] = Z[slot 2q+j, m][o]
            osb = constp.tile([36, Q * D_OUT], f32, name="osb")

            def load_ring(t):
                return (nc.sync, nc.scalar)[t % 2]

            # Emit the prefetch-window load issues before the compute loop:
            # every engine-queue entry ahead of them is wait-free, so both
            # rings stream back-to-back from the start.
            eb_tiles = {}
            for t in range(min(PREFETCH, K)):
                ebt = ep.tile([P, H], bf16)
                if t == 0:
                    # split the first load+multiply so the first matmuls'
                    # region deps fire after 0.25 MiB instead of 0.5 MiB
                    nc.sync.dma_start(ebt[:, 0 : H // 2], eg[0, :, 0 : H // 2])
                    nc.sync.dma_start(ebt[:, H // 2 : H], eg[0, :, H // 2 : H])
                else:
                    load_ring(t).dma_start(ebt[:], eg[t])
                eb_tiles[t] = ebt

            qhalf = Q // 2
            wep_tiles = []
            for t in range(K):
                q, j = t // 2, t % 2
                npair = min(2, K - q * 2)
                # issue the load that keeps the prefetch window full
                tp = t + PREFETCH
                if tp < K:
                    ebt = ep.tile([P, H], bf16)
                    load_ring(tp).dma_start(ebt[:], eg[tp])
                    eb_tiles[tp] = ebt

                ebt = eb_tiles.pop(t)
                wep_t = wep.tile([P, H], bf16)
                if t == 0:
                    nc.vector.tensor_mul(
                        wep_t[:, 0 : H // 2], ebt[:, 0 : H // 2], w_b[:, 0 : H // 2]
                    )
                    nc.vector.tensor_mul(
                        wep_t[:, H // 2 : H], ebt[:, H // 2 : H], w_b[:, H // 2 : H]
                    )
                else:
                    nc.vector.tensor_mul(wep_t[:], ebt[:], w_b[:])
                wep_tiles.append(wep_t)

                if j != npair - 1:
                    continue

                # c-interleave the pair's matmuls so both column-group
                # streams run concurrently in the PE array
                ps = psp.tile([64, D_OUT], f32)
                for c in range(C):
                    for jj in range(npair):
                        tt = q * 2 + jj
                        nc.tensor.matmul(
                            ps[32 * jj : 32 * jj + M, :],
                            x_b[:, (c * K + tt) * M : (c * K + tt) * M + M],
                            wep_tiles[jj][:, c * D_OUT : (c + 1) * D_OUT],
                            start=(c == 0),
                            stop=(not with_bias and c == C - 1),
                        )
                if with_bias:
                    for jj in range(npair):
                        tt = q * 2 + jj
                        nc.tensor.matmul(
                            ps[32 * jj : 32 * jj + M, :],
                            ones_b[:],
                            mbrow[0:1, tt * D_OUT : (tt + 1) * D_OUT],
                            start=False,
                            stop=True,
                        )
                # drain the pair on the otherwise idle ScalarE
                for jj in range(npair):
                    nc.scalar.copy(
                        osb[32 * jj : 32 * jj + M, q * D_OUT : (q + 1) * D_OUT],
                        ps[32 * jj : 32 * jj + M, :],
                    )
                wep_tiles = []

                # store the first half of the staging tile mid-run
                if q == qhalf - 1:
                    for jj in range(2):
                        nc.sync.dma_start(
                            out[jj, :, 0:qhalf, :],
                            osb[32 * jj : 32 * jj + M, 0 : qhalf * D_OUT],
                        )
            for jj in range(2):
                load_ring(jj).dma_start(
                    out[jj, :, qhalf:Q, :],
                    osb[32 * jj : 32 * jj + M, qhalf * D_OUT : Q * D_OUT],
                )

    nc.compile()
    return nc


def _pack(idx):
    """Group samples by bank, pack bank-slots onto cores.

    Returns (K, plan): plan[c] is a list of (bank, [samples]) slots, each
    carrying at most M samples of one bank; K = max slots per core.
    """
    from collections import defaultdict

    groups = defaultdict(list)
    for s, b in enumerate(idx):
        groups[int(b)].append(s)
    slots = []
    for b, ss in groups.items():
        for i in range(0, len(ss), M):
            slots.append((b, ss[i : i + M]))
    slots.sort(key=lambda x: -len(x[1]))
    plan = [[] for _ in range(N_CORES)]
    for b, ss in slots:
        c = min(range(N_CORES), key=lambda c: len(plan[c]))
        plan[c].append((b, ss))
    K = max(len(p) for p in plan)
    return K, plan


def _install_trace_shim():
    """Register the axon NTFF profile hook bass_utils expects (the agent
    image lacks antenv.axon_hooks; the C ABI is in libaxon_pjrt.so)."""
    import contextlib
    import ctypes
    import sys
    import types

    if "antenv.axon_hooks" in sys.modules:
        return
    lib = ctypes.CDLL("/opt/axon/libaxon_pjrt.so")
    if not hasattr(lib, "axon_start_nrt_profile"):
        hook = None
    else:
        lib.axon_start_nrt_profile.argtypes = [
            ctypes.POINTER(ctypes.c_int64),
            ctypes.c_size_t,
        ]
        lib.axon_start_nrt_profile.restype = ctypes.c_int64
        lib.axon_stop_nrt_profile.argtypes = [ctypes.c_char_p]
        lib.axon_stop_nrt_profile.restype = ctypes.c_int64

        @contextlib.contextmanager
        def hook(output_dir, device_ids):
            import jax

            jax.devices()
            if device_ids:
                ids = (ctypes.c_int64 * len(device_ids))(*device_ids)
                rc = lib.axon_start_nrt_profile(ids, len(device_ids))
            else:
                rc = lib.axon_start_nrt_profile(None, 0)
            if rc != 0:
                raise RuntimeError(f"axon_start_nrt_profile rc={rc}")
            try:
                yield
            finally:
                n = lib.axon_stop_nrt_profile(str(output_dir).encode())
                print(f"ntff profile: {n} file(s) -> {output_dir}", file=sys.stderr)

    mod = types.ModuleType("antenv.axon_hooks")
    mod.get_axon_ntff_profile_hook = lambda: hook
    mod.set_axon_ntff_profile_hook = lambda h: None
    sys.modules["antenv.axon_hooks"] = mod


def kernel(X, W, bias, Werr_bank, Berr_bank, idx):
    global last_exec_time_ns
    import os

    import ml_dtypes

    from concourse.bass_utils import run_bass_kernel_spmd

    bf16 = ml_dtypes.bfloat16
    X = np.asarray(X, dtype=np.float32)
    W = np.asarray(W, dtype=np.float32)
    bias = np.asarray(bias, dtype=np.float32)
    Werr_bank = np.asarray(Werr_bank, dtype=np.float32)
    Berr_bank = np.asarray(Berr_bank, dtype=np.float32)
    idx = np.asarray(idx, dtype=np.int32)

    K, plan = _pack(idx)
    Q = -(-K // 2)
    R = K * M
    with_bias = bool(np.any(bias))
    if ("nc", K, with_bias) not in _CACHE:
        _CACHE[("nc", K, with_bias)] = _build_nc(K, with_bias)
    nc = _CACHE[("nc", K, with_bias)]

    wt = np.ascontiguousarray(
        W.astype(bf16).reshape(C, P, D_OUT).transpose(1, 0, 2).reshape(P, C * D_OUT)
    )

    in_maps = []
    row_of_sample = np.full(B, -1, dtype=np.int64)
    for c_id in range(N_CORES):
        slots = plan[c_id]
        banks = [b for b, _ in slots] + [0] * (K - len(slots))
        eg = np.ascontiguousarray(
            Werr_bank[banks]
            .astype(bf16)
            .reshape(K, C, P, D_OUT)
            .transpose(0, 2, 1, 3)
            .reshape(K, P, C * D_OUT)
        )
        xs = np.zeros((R, D_IN), dtype=np.float32)
        for t, (b, ss) in enumerate(slots):
            q, j = t // 2, t % 2
            for m, s in enumerate(ss):
                xs[t * M + m] = X[s]
                # device row (j, m, q) of out [2, M, Q, 512]
                row_of_sample[s] = c_id * (2 * M * Q) + (j * M + m) * Q + q
        xtt = np.ascontiguousarray(
            xs.T.astype(bf16).reshape(C, P, R).transpose(1, 0, 2).reshape(P, C * R)
        )
        im = {"eg": eg, "wt": wt, "xtt": xtt}
        if with_bias:
            im["bb"] = np.ascontiguousarray(
                np.broadcast_to(bias.reshape(1, D_OUT), (K, D_OUT))
            )
            im["beg"] = np.ascontiguousarray(Berr_bank[banks, 0, :])
        in_maps.append(im)
    assert (row_of_sample >= 0).all()

    trace = os.environ.get("BASS_KERNEL_TRACE") == "1"
    if trace:
        _install_trace_shim()
    res = run_bass_kernel_spmd(
        nc,
        in_maps,
        core_ids=list(range(N_CORES)),
        trace=trace,
        trace_cores=(
            list(range(N_CORES))
            if os.environ.get("BASS_KERNEL_TRACE_ALL") == "1"
            else [0]
        )
        if trace
        else None,
    )
    last_exec_time_ns = res.exec_time_ns
    allrows = np.concatenate(
        [r["out"].reshape(2 * M * Q, D_OUT) for r in res.results], axis=0
    )
    return np.ascontiguousarray(allrows[row_of_sample])
